# revision 29
# baseline (speedup 1.0000x reference)
"""Trainium2 Bass kernel: BertUnpadSelfAttention (B=8, S=1024, H=12, D=64).

Strategy (v2)
-------------
Shard by (batch, head) pairs instead of batch: the sequence lengths vary
(512..1024), so batch-per-core sharding makes every core pay for the
longest batch.  96 (b,h) pairs are grouped into length-classes and
bin-packed into an identical per-core slot schedule (same instruction
stream on all 8 cores; the host packs each core's pairs into the slots).
Masked work is skipped entirely: k-chunks and q-chunks beyond L are never
computed and their exp(bias) tiles are never streamed.

Device (per core, fp16 matmuls -> fp32 PSUM):
  * proj per pair: qk^T = W_qk^T @ hT -> [128 = q64|k64 dims, L tokens];
    v = hT^T @ W_v -> [token, head, d] with a ones column for the softmax
    denominator
  * per slot (pair, q-chunk<=512), per k-chunk of 128:
    scores^T [128k, qlen] = kT.T @ qT (one matmul), exp on ScalarE,
    multiply by host-precomputed exp(bias)*2^-4 tile on VectorE,
    then reverse-PV: att[q<=128, d|sum] += p^T.T @ [v|1]  (probs as the
    stationary operand: 65-row streams instead of 512 -> half the PE time)
  * proj of the next class is interleaved into the attention k-loop;
    PSUM->SBUF evacuations run on GpSimd (otherwise idle)
  * Eb streams via few large descriptor-rich DMAs issued from GpSimd

Host: pack per-core hT/W/Eb; post: divide by denominator, write rows
directly at cu_seqlens offsets (valid tokens are contiguous per batch).
"""

import numpy as np

B, S, H, D = 8, 1024, 12, 64
HID = H * D            # 768
NC = 8                 # cores
KCH = 128              # k chunk
QCH = 512              # max q chunk / moving free dim
EB_SCALE = 0.0625      # folded into exp(bias); cancels in softmax

_CACHE = {}


# --------------------------------------------------------------------------
# schedule
# --------------------------------------------------------------------------

class _Cls:
    __slots__ = ("L", "Lp", "nk", "np_", "ngrp", "ht_off", "vv_base",
                 "qk_offs", "w_blocks", "v_off", "qslots", "pairs_percore")


class _Slot:
    __slots__ = ("ci", "pl", "qoff", "qlen", "nsub", "eb_col", "out_off")


def _build_schedule(lens):
    """Uniform per-core slot schedule from the 8 sequence lengths."""
    lens = [int(x) for x in lens]
    assert len(lens) == B and all(0 < l <= S for l in lens)

    # merge batches with equal L into one class; if a core's slice of a
    # class would straddle two batches, fall back to one class per batch
    def mk_classes(group_by_len):
        if group_by_len:
            ls = sorted(set(lens), reverse=True)
            return [(L, [b for b in range(B) if lens[b] == L]) for L in ls]
        order = sorted(range(B), key=lambda b: -lens[b])
        return [(lens[b], [b]) for b in order]

    for group_by_len in (True, False):
        classes = []
        ok = True
        for L, batches in mk_classes(group_by_len):
            pairs = [(b, h) for b in batches for h in range(H)]
            np_ = -(-len(pairs) // NC)
            padded = pairs + [None] * (np_ * NC - len(pairs))
            percore = [padded[c * np_:(c + 1) * np_] for c in range(NC)]
            for c in range(NC):
                bs = {p[0] for p in percore[c] if p is not None}
                if len(bs) > 1:
                    ok = False
            classes.append((L, percore, np_))
        if ok:
            break
    assert ok, "schedule fallback failed"

    # W group-block dedupe: pairs are processed in groups of 2 (two swapped
    # projection passes [qA|kB], [kA|qB] so both operands of the scores
    # matmul share a base partition).  A group block can be reused if on
    # every core the (headA, headB) at that block matches (or is pad).
    block_heads = [dict() for _ in range(NC)]   # per core: block -> (hA, hB)
    nblocks = 0
    out_classes = []
    ht_off = vv_base = qk_off = 0
    for L, percore, np_ in classes:
        nk = -(-L // KCH)
        Lp = nk * KCH
        ngrp = -(-np_ // 2)
        cls = _Cls()
        cls.L, cls.Lp, cls.nk, cls.np_, cls.ngrp = L, Lp, nk, np_, ngrp
        cls.pairs_percore = percore
        cls.ht_off = ht_off
        cls.vv_base = vv_base
        cls.qk_offs = []
        cls.w_blocks = []

        def _grp_heads(c, g):
            pa = percore[c][2 * g]
            pb = percore[c][2 * g + 1] if 2 * g + 1 < np_ else None
            return (pa[1] if pa is not None else None,
                    pb[1] if pb is not None else None)

        for g in range(ngrp):
            hs = [_grp_heads(c, g) for c in range(NC)]

            def _compat(cand):
                for c, (ha, hb) in enumerate(hs):
                    cur = block_heads[c].get(cand)
                    if cur is not None:
                        if ha is not None and cur[0] is not None and cur[0] != ha:
                            return False
                        if hb is not None and cur[1] is not None and cur[1] != hb:
                            return False
                return True

            beta = None
            for cand in range(nblocks):
                if _compat(cand):
                    beta = cand
                    break
            if beta is None:
                beta = nblocks
                nblocks += 1
            for c, (ha, hb) in enumerate(hs):
                cur = block_heads[c].get(beta, (None, None))
                block_heads[c][beta] = (ha if ha is not None else cur[0],
                                        hb if hb is not None else cur[1])
            cls.w_blocks.append(beta)
            cls.qk_offs.append((qk_off, qk_off + Lp))
            qk_off += 2 * Lp
        cls.qslots = []
        q = 0
        while q < L:
            cls.qslots.append((q, min(QCH, L - q)))
            q += QCH
        ht_off += Lp
        vv_base += nk * np_
        out_classes.append(cls)

    slots = []
    eb_col = out_off = 0
    for ci, cls in enumerate(out_classes):
        for pl in range(cls.np_):
            for (qoff, qlen) in cls.qslots:
                sl = _Slot()
                sl.ci, sl.pl, sl.qoff, sl.qlen = ci, pl, qoff, qlen
                sl.nsub = -(-qlen // 128)
                sl.eb_col = eb_col
                sl.out_off = out_off
                eb_col += cls.nk * qlen
                out_off += sl.nsub
                slots.append(sl)

    # v strips appended after the qk blocks (contiguous per class)
    v_off = nblocks * 256
    for cls in out_classes:
        cls.v_off = v_off
        v_off += cls.np_ * 64

    sched = {
        "classes": out_classes,
        "slots": slots,
        "WTOT": v_off,
        "NW": nblocks,
        "HT_COLS": ht_off,
        "QKT_COLS": qk_off,
        "VCH": vv_base,
        "EB_COLS": eb_col,
        "NSUB": out_off,
        "lens": lens,
    }
    return sched


def _get_sched(lens_key):
    key = ("sched", lens_key)
    if key not in _CACHE:
        _CACHE[key] = _build_schedule(list(lens_key))
    return _CACHE[key]


# --------------------------------------------------------------------------
# device program
# --------------------------------------------------------------------------

def _build_nc(sched, use_bias):
    import concourse.mybir as mybir
    import concourse.tile as tile
    from concourse import bacc

    f16 = mybir.dt.float16

    nc = bacc.Bacc("TRN2", debug=False, num_devices=NC)
    # partition-major layouts: row p holds partition p's data contiguously
    # (large DMA descriptors). hT/W carry an extra 129th row for the bias.
    hT = nc.dram_tensor("hT", [129, 6 * sched["HT_COLS"]], f16,
                        kind="ExternalInput").ap()
    W = nc.dram_tensor("W", [129, 6 * sched["WTOT"]], f16,
                       kind="ExternalInput").ap()
    Eb = nc.dram_tensor("Eb", [128, sched["EB_COLS"]], f16,
                        kind="ExternalInput").ap()
    out = nc.dram_tensor("out", [128, sched["NSUB"] * 65], f16,
                         kind="ExternalOutput").ap()

    with tile.TileContext(nc) as tc:
        _emit_body(nc, tc, tile, mybir, hT, W, Eb, out, sched, use_bias)
    nc.compile()
    return nc


def _emit_body(nc, tc, tile, mybir, hT, W, Eb, out, sched, use_bias):
    f16 = mybir.dt.float16
    f32 = mybir.dt.float32
    Exp = mybir.ActivationFunctionType.Exp
    classes = sched["classes"]
    slots = sched["slots"]
    NW = sched["NW"]

    with (
        tc.tile_pool(name="per", bufs=1) as per,
        tc.tile_pool(name="ebp", bufs=3) as ebp,
        tc.tile_pool(name="ste", bufs=3) as ste,
        tc.tile_pool(name="stp", bufs=9) as stp,
        tc.tile_pool(name="osb", bufs=3) as osb,
        tc.tile_pool(name="psc", bufs=2, space="PSUM") as psc,
        tc.tile_pool(name="pat", bufs=2, space="PSUM") as pat,
        tc.tile_pool(name="pj", bufs=2, space="PSUM") as pj,
    ):
        # ---- persistent tiles ------------------------------------------
        # W: qk group blocks (256 cols: [qA|kB][kA|qB]) then v strips
        hT_sb = per.tile([128, 6, sched["HT_COLS"]], f16)
        W_sb = per.tile([128, 6, sched["WTOT"]], f16)
        qkT = per.tile([128, sched["QKT_COLS"]], f16)
        vv = per.tile([128, sched["VCH"], 65], f16)
        nc.vector.memset(vv[:, :, 64:65], 1.0)
        HTC = sched["HT_COLS"]
        WC = sched["WTOT"]
        if use_bias:
            hT_last = per.tile([1, 6, HTC], f16)
            W_last2 = per.tile([1, 6, WC], f16)
            nc.sync.dma_start(
                hT_last, hT[128:129, :].rearrange("o (i c) -> o i c", c=HTC)
            )
            nc.sync.dma_start(
                W_last2, W[128:129, :].rearrange("o (i c) -> o i c", c=WC)
            )

        # ---- upfront DMAs -----------------------------------------------
        # class 0 + W: per-ic chunks interleaved on SP so the first
        # projection chain starts after ~0.25MB; later classes: one strided
        # DMA each on the scalar queue (latency hidden by class-0 work)
        hT_src = hT[0:128, :].rearrange("p (i c) -> p i c", c=HTC)
        W_src = W[0:128, :].rearrange("p (i c) -> p i c", c=WC)

        c0 = classes[0].ht_off
        L0 = classes[0].Lp
        for ic in range(6):
            nc.sync.dma_start(
                hT_sb[:, ic, c0:c0 + L0], hT_src[:, ic, c0:c0 + L0]
            )
            nc.sync.dma_start(W_sb[:, ic], W_src[:, ic])
        for cls in classes[1:]:
            for ic in range(6):
                nc.sync.dma_start(
                    hT_sb[:, ic, cls.ht_off:cls.ht_off + cls.Lp],
                    hT_src[:, ic, cls.ht_off:cls.ht_off + cls.Lp],
                )

        # ---- projection job closures -----------------------------------
        def qk_half(cls, g, pss, lc, box, ics):
            # pss 0: W cols [0:128] = [qA|kB]; pss 1: [128:256] = [kA|qB]
            cw = min(QCH, cls.Lp - lc * QCH)
            c0 = cls.w_blocks[g] * 256 + 128 * pss
            dst = cls.qk_offs[g][pss] + lc * QCH
            if ics.start == 0:
                box["ps"] = pj.tile([128, QCH], f32, tag="pj", name="ps_qk")
            ps = box["ps"]
            last = ics.stop == 6
            for ic in ics:
                nc.tensor.matmul(
                    ps[:, :cw],
                    W_sb[:, ic, c0:c0 + 128],
                    hT_sb[:, ic, cls.ht_off + lc * QCH:
                          cls.ht_off + lc * QCH + cw],
                    start=(ic == 0),
                    stop=(ic == 5 and last and not use_bias),
                )
            if last:
                if use_bias:
                    nc.tensor.matmul(
                        ps[:, :cw],
                        W_last2[:, 0, c0:c0 + 128],
                        hT_last[:, 0, cls.ht_off + lc * QCH:
                                cls.ht_off + lc * QCH + cw],
                        start=False, stop=True,
                    )
                nc.vector.tensor_copy(qkT[:, dst:dst + cw], ps[:, :cw])

        def qk_job(cls, g, pss, lc):
            def run():
                qk_half(cls, g, pss, lc, {}, range(0, 6))
            return run

        def v_job(cls, kc):
            def run():
                npr = cls.np_
                n = npr * 64
                ps = pj.tile([128, QCH], f32, tag="pj", name="ps_v")
                for ic in range(6):
                    nc.tensor.matmul(
                        ps[:, :n],
                        hT_sb[:, ic, cls.ht_off + kc * KCH:
                              cls.ht_off + (kc + 1) * KCH],
                        W_sb[:, ic, cls.v_off:cls.v_off + n],
                        start=(ic == 0), stop=(ic == 5 and not use_bias),
                    )
                if use_bias:
                    nc.tensor.matmul(
                        ps[:, :n],
                        hT_last[:, 0, cls.ht_off + kc * KCH:
                                cls.ht_off + (kc + 1) * KCH],
                        W_last2[:, 0, cls.v_off:cls.v_off + n],
                        start=False, stop=True,
                    )
                nc.vector.tensor_copy(
                    vv[:, cls.vv_base + kc * npr:
                       cls.vv_base + (kc + 1) * npr, 0:64],
                    ps[:, :n].rearrange("p (h d) -> p h d", d=64),
                )
            return run

        def proj_jobs(ci):
            cls = classes[ci]
            jobs = []
            for g in range(cls.ngrp):
                for pss in range(2):
                    for lc in range(-(-cls.Lp // QCH)):
                        jobs.append(qk_job(cls, g, pss, lc))
            for kc in range(cls.nk):
                jobs.append(v_job(cls, kc))
            return jobs

        # ---- Eb prefetch ------------------------------------------------
        eb_tiles = {}

        def issue_eb(si):
            sl = slots[si]
            cls = classes[sl.ci]
            t = ebp.tile([128, 8, QCH], f16, tag="eb", name="eb")
            eb_tiles[si] = t
            nk1 = cls.nk // 2
            for (k0, k1) in ((0, nk1), (nk1, cls.nk)):
                if k1 <= k0:
                    continue
                c0 = sl.eb_col + k0 * sl.qlen
                nc.sync.dma_start(
                    t[:, k0:k1, :sl.qlen],
                    Eb[:, c0:c0 + (k1 - k0) * sl.qlen].rearrange(
                        "p (n q) -> p n q", q=sl.qlen
                    ),
                )

        # ---- prologue ---------------------------------------------------
        # class-0 qk chains as interleaved half-chains (ics 0-2 then 3-5)
        # so the PE starts before the later ic-chunk DMAs have landed
        cls0 = classes[0]
        qk_keys = [(g, pss, lc)
                   for g in range(cls0.ngrp) for pss in range(2)
                   for lc in range(-(-cls0.Lp // QCH))]
        for i in range(0, len(qk_keys), 2):
            group = qk_keys[i:i + 2]
            boxes = [dict() for _ in group]
            for (g, pss, lc), box in zip(group, boxes):
                qk_half(cls0, g, pss, lc, box, range(0, 3))
            for (g, pss, lc), box in zip(group, boxes):
                qk_half(cls0, g, pss, lc, box, range(3, 6))
        for kc in range(cls0.nk):
            v_job(cls0, kc)()
        issue_eb(0)
        if len(slots) > 1:
            issue_eb(1)

        # ---- main loop --------------------------------------------------
        # software pipeline: scores/exp/mul for slot i run while the PV
        # matmuls for slot i-1 stream (each q-sub's PSUM accumulation group
        # is sequential in its own bank: start=True resets the whole bank)
        def emit_pv_all(pend):
            (pts, cls_p, pl_p, qlen_p, nsub_p, out_off_p) = pend
            ob = osb.tile([128, 4, 65], f16, tag="ob", name="ob")
            for sub in range(nsub_p):
                qn = min(128, qlen_p - sub * 128)
                att = pat.tile([128, 128], f32, tag="att", name="att",
                               padded_shape=[128, QCH])
                for kc in range(cls_p.nk):
                    nc.tensor.matmul(
                        att[0:qn, 0:65],
                        pts[kc // 2][:, kc % 2, sub * 128:sub * 128 + qn],
                        vv[:, cls_p.vv_base + kc * cls_p.np_ + pl_p, :],
                        start=(kc == 0), stop=(kc == cls_p.nk - 1),
                    )
                nc.vector.tensor_copy(ob[:, sub, :], att[:, 0:65])
            nc.sync.dma_start(
                out[:, out_off_p * 65:(out_off_p + nsub_p) * 65].rearrange(
                    "p (n x) -> p n x", x=65
                ),
                ob[:, :nsub_p, :],
            )

        si = 0
        pending = None
        for ci, cls in enumerate(classes):
            fillers = proj_jobs(ci + 1) if ci + 1 < len(classes) else []
            n_iters = cls.np_ * len(cls.qslots) * cls.nk
            stride = max(1, n_iters // max(1, len(fillers)))
            it = 0
            fi = 0
            for pl in range(cls.np_):
                for (qoff, qlen) in cls.qslots:
                    sl = slots[si]
                    eb = eb_tiles.pop(si)
                    g, half = pl // 2, pl % 2
                    off1, off2 = cls.qk_offs[g]
                    # half 0: q in T1[0:64],  k in T2[0:64]
                    # half 1: q in T2[64:128], k in T1[64:128]
                    p0 = 64 * half
                    koff = off1 if half else off2
                    qoff_t = off2 if half else off1

                    pts = []
                    for kg in range(0, cls.nk, 2):
                        n2 = min(2, cls.nk - kg)
                        sps = psc.tile([128, 2, QCH], f32, tag="sc",
                                       name="sps")
                        for j in range(n2):
                            kc = kg + j
                            nc.tensor.matmul(
                                sps[:, j, :qlen],
                                qkT[p0:p0 + 64,
                                    koff + kc * KCH:koff + (kc + 1) * KCH],
                                qkT[p0:p0 + 64,
                                    qoff_t + qoff:qoff_t + qoff + qlen],
                                start=True, stop=True,
                            )
                            if (fillers and fi < len(fillers)
                                    and it % stride == 0):
                                fillers[fi]()
                                fi += 1
                            it += 1
                        es = ste.tile([128, 2, QCH], f16, tag="es", name="es")
                        nc.scalar.activation(
                            es[:, :n2, :qlen], sps[:, :n2, :qlen], Exp,
                            scale=0.125,
                        )
                        pt = stp.tile([128, 2, QCH], f16, tag="pt", name="pt")
                        mul_eng = (nc.gpsimd if (kg // 2) % 4 == 3 else nc.vector)
                        mul_eng.tensor_mul(
                            pt[:, :n2, :qlen], es[:, :n2, :qlen],
                            eb[:, kg:kg + n2, :qlen]
                        )
                        pts.append(pt)
                    if pending is not None:
                        emit_pv_all(pending)
                    pending = (pts, cls, pl, qlen, sl.nsub, sl.out_off)
                    if si + 2 < len(slots):
                        issue_eb(si + 2)
                    si += 1
            while fi < len(fillers):
                fillers[fi]()
                fi += 1
        emit_pv_all(pending)


def _get_nc(lens_key, use_bias):
    key = ("nc", lens_key, use_bias)
    if key not in _CACHE:
        _CACHE[key] = _build_nc(_get_sched(lens_key), use_bias)
    return _CACHE[key]


# --------------------------------------------------------------------------
# host pack / unpack
# --------------------------------------------------------------------------

def prepare_in_maps(inputs):
    hidden = np.asarray(inputs["hidden_states"], np.float32)
    Wf = np.asarray(inputs["Wqkv_w"], np.float32)
    bvec = np.asarray(inputs["Wqkv_b"], np.float32)
    bias = np.asarray(inputs["bias"], np.float32)
    indices = np.asarray(inputs["indices"], np.int32)
    cu = np.asarray(inputs["cu_seqlens"], np.int64)
    lens = np.diff(cu).astype(np.int64)
    nnz = hidden.shape[0]

    # valid tokens must be the first L of each batch row-block
    expect = np.concatenate(
        [b * S + np.arange(l) for b, l in enumerate(lens)]
    ) if len(lens) == B else None
    contiguous = (
        expect is not None
        and indices.shape[0] == expect.shape[0]
        and np.array_equal(indices, expect)
    )
    if not contiguous:
        # fallback: dense full-length processing, scatter rows
        lens = np.full(B, S, np.int64)
        hp = np.zeros((B * S, HID), np.float32)
        hp[indices] = hidden
        tok = [hp[b * S:(b + 1) * S] for b in range(B)]
    else:
        tok = [hidden[cu[b]:cu[b + 1]] for b in range(B)]

    lens_key = tuple(int(x) for x in lens)
    sched = _get_sched(lens_key)
    use_bias = bool(np.any(bvec != 0.0))

    # 1/sqrt(D) is applied via the Exp activation's scale parameter
    Ws = Wf
    bs = bvec

    classes = sched["classes"]
    slots = sched["slots"]
    NW = sched["NW"]

    HTC = sched["HT_COLS"]

    WTOT = sched["WTOT"]

    def prep_core(c):
        hTa = np.zeros((HID + 1, HTC), np.float16)
        hTa[HID] = 1.0
        Wd = np.zeros((HID + 1, WTOT), np.float16)
        Ebd = np.zeros((128, sched["EB_COLS"]), np.float16)
        for cls in classes:
            batches = {p[0] for p in cls.pairs_percore[c] if p is not None}
            if batches:
                b0 = next(iter(batches))
                L = int(lens[b0])
                hTa[0:HID, cls.ht_off:cls.ht_off + L] = tok[b0].T
            for pl, pair in enumerate(cls.pairs_percore[c]):
                if pair is None:
                    continue
                _, h = pair
                beta = cls.w_blocks[pl // 2]
                half = pl % 2
                # qk block (256): [qA|kB][kA|qB]; v strip: v_off + pl*64
                qc0 = beta * 256 + (192 if half else 0)
                kc0 = beta * 256 + (64 if half else 128)
                vc0 = cls.v_off + pl * 64
                Wd[0:HID, qc0:qc0 + 64] = Ws[:, h * D:(h + 1) * D]
                Wd[0:HID, kc0:kc0 + 64] = Ws[:, HID + h * D:HID + (h + 1) * D]
                Wd[0:HID, vc0:vc0 + 64] = \
                    Ws[:, 2 * HID + h * D:2 * HID + (h + 1) * D]
                Wd[HID, qc0:qc0 + 64] = bs[h * D:(h + 1) * D]
                Wd[HID, kc0:kc0 + 64] = bs[HID + h * D:HID + (h + 1) * D]
                Wd[HID, vc0:vc0 + 64] = \
                    bs[2 * HID + h * D:2 * HID + (h + 1) * D]
        with np.errstate(under="ignore"):
            for sl in slots:
                cls = classes[sl.ci]
                pair = cls.pairs_percore[c][sl.pl]
                if pair is None:
                    continue
                b, h = pair
                L = int(lens[b])
                sub = bias[b, h, sl.qoff:sl.qoff + sl.qlen, 0:L]
                arr = np.zeros((cls.nk * KCH, sl.qlen), np.float16)
                arr[:L] = (np.exp(sub) * EB_SCALE).T.astype(np.float16)
                Ebd[:, sl.eb_col:sl.eb_col + cls.nk * sl.qlen] = (
                    arr.reshape(cls.nk, 128, sl.qlen)
                    .transpose(1, 0, 2).reshape(128, cls.nk * sl.qlen)
                )
        # partition-major repack: row p holds its 6 ic chunks contiguously
        hT_pm = np.zeros((129, 6 * HTC), np.float16)
        hT_pm[0:128] = (hTa[0:HID].reshape(6, 128, HTC)
                        .transpose(1, 0, 2).reshape(128, 6 * HTC))
        hT_pm[128, 0:HTC] = hTa[HID]
        W_pm = np.zeros((129, 6 * WTOT), np.float16)
        W_pm[0:128] = (Wd[0:HID].reshape(6, 128, WTOT)
                       .transpose(1, 0, 2).reshape(128, 6 * WTOT))
        W_pm[128, 0:WTOT] = Wd[HID]
        return {"hT": hT_pm, "W": W_pm, "Eb": Ebd}

    from concurrent.futures import ThreadPoolExecutor
    with ThreadPoolExecutor(max_workers=8) as ex:
        in_maps = list(ex.map(prep_core, range(NC)))

    meta = {
        "lens_key": lens_key,
        "cu": cu,
        "nnz": nnz,
        "contiguous": contiguous,
        "indices": indices,
    }
    return in_maps, meta, use_bias


def postprocess(results, meta):
    sched = _get_sched(meta["lens_key"])
    classes = sched["classes"]
    slots = sched["slots"]
    cu = meta["cu"]
    if meta["contiguous"]:
        out_full = np.zeros((meta["nnz"], HID), np.float32)
    else:
        out_full = np.zeros((B * S, HID), np.float32)
    for c in range(NC):
        o = np.asarray(results[c]["out"], np.float32)   # [128, NSUB*65]
        o = o.reshape(128, sched["NSUB"], 65).transpose(1, 0, 2)
        for sl in slots:
            cls = classes[sl.ci]
            pair = cls.pairs_percore[c][sl.pl]
            if pair is None:
                continue
            b, h = pair
            base = (cu[b] if meta["contiguous"] else b * S)
            for sub in range(sl.nsub):
                qn = min(128, sl.qlen - sub * 128)
                blk = o[sl.out_off + sub, :qn]
                att = blk[:, :64] / blk[:, 64:65]
                r0 = base + sl.qoff + sub * 128
                out_full[r0:r0 + qn, h * D:(h + 1) * D] = att
    if not meta["contiguous"]:
        out_full = out_full[meta["indices"]]
    return out_full


def _run_spmd(in_maps, meta, use_bias=True, trace=False):
    from concourse.bass_utils import run_bass_kernel_spmd
    return run_bass_kernel_spmd(
        _get_nc(meta["lens_key"], use_bias), in_maps,
        core_ids=list(range(NC)), trace=trace,
    )


def kernel(**inputs):
    in_maps, meta, use_bias = prepare_in_maps(inputs)
    res = _run_spmd(in_maps, meta, use_bias=use_bias)
    return postprocess(res.results, meta)


# revision 30
# speedup vs baseline: 1.0099x; 1.0099x over previous
"""Trainium2 Bass kernel: BertUnpadSelfAttention (B=8, S=1024, H=12, D=64).

Strategy (v2)
-------------
Shard by (batch, head) pairs instead of batch: the sequence lengths vary
(512..1024), so batch-per-core sharding makes every core pay for the
longest batch.  96 (b,h) pairs are grouped into length-classes and
bin-packed into an identical per-core slot schedule (same instruction
stream on all 8 cores; the host packs each core's pairs into the slots).
Masked work is skipped entirely: k-chunks and q-chunks beyond L are never
computed and their exp(bias) tiles are never streamed.

Device (per core, fp16 matmuls -> fp32 PSUM):
  * proj per pair: qk^T = W_qk^T @ hT -> [128 = q64|k64 dims, L tokens];
    v = hT^T @ W_v -> [token, head, d] with a ones column for the softmax
    denominator
  * per slot (pair, q-chunk<=512), per k-chunk of 128:
    scores^T [128k, qlen] = kT.T @ qT (one matmul), exp on ScalarE,
    multiply by host-precomputed exp(bias)*2^-4 tile on VectorE,
    then reverse-PV: att[q<=128, d|sum] += p^T.T @ [v|1]  (probs as the
    stationary operand: 65-row streams instead of 512 -> half the PE time)
  * proj of the next class is interleaved into the attention k-loop;
    PSUM->SBUF evacuations run on GpSimd (otherwise idle)
  * Eb streams via few large descriptor-rich DMAs issued from GpSimd

Host: pack per-core hT/W/Eb; post: divide by denominator, write rows
directly at cu_seqlens offsets (valid tokens are contiguous per batch).
"""

import numpy as np

B, S, H, D = 8, 1024, 12, 64
HID = H * D            # 768
NC = 8                 # cores
KCH = 128              # k chunk
QCH = 512              # max q chunk / moving free dim
EB_SCALE = 0.0625      # folded into exp(bias); cancels in softmax

_CACHE = {}


# --------------------------------------------------------------------------
# schedule
# --------------------------------------------------------------------------

class _Cls:
    __slots__ = ("L", "Lp", "nk", "np_", "ngrp", "ht_off", "vv_base",
                 "qk_offs", "w_blocks", "v_off", "qslots", "pairs_percore")


class _Slot:
    __slots__ = ("ci", "pl", "qoff", "qlen", "nsub", "eb_col", "out_off")


def _build_schedule(lens):
    """Uniform per-core slot schedule from the 8 sequence lengths."""
    lens = [int(x) for x in lens]
    assert len(lens) == B and all(0 < l <= S for l in lens)

    # merge batches with equal L into one class; if a core's slice of a
    # class would straddle two batches, fall back to one class per batch
    def mk_classes(group_by_len):
        if group_by_len:
            ls = sorted(set(lens), reverse=True)
            return [(L, [b for b in range(B) if lens[b] == L]) for L in ls]
        order = sorted(range(B), key=lambda b: -lens[b])
        return [(lens[b], [b]) for b in order]

    for group_by_len in (True, False):
        classes = []
        ok = True
        for L, batches in mk_classes(group_by_len):
            pairs = [(b, h) for b in batches for h in range(H)]
            np_ = -(-len(pairs) // NC)
            padded = pairs + [None] * (np_ * NC - len(pairs))
            percore = [padded[c * np_:(c + 1) * np_] for c in range(NC)]
            for c in range(NC):
                bs = {p[0] for p in percore[c] if p is not None}
                if len(bs) > 1:
                    ok = False
            classes.append((L, percore, np_))
        if ok:
            break
    assert ok, "schedule fallback failed"

    # W group-block dedupe: pairs are processed in groups of 2 (two swapped
    # projection passes [qA|kB], [kA|qB] so both operands of the scores
    # matmul share a base partition).  A group block can be reused if on
    # every core the (headA, headB) at that block matches (or is pad).
    block_heads = [dict() for _ in range(NC)]   # per core: block -> (hA, hB)
    nblocks = 0
    out_classes = []
    ht_off = vv_base = qk_off = 0
    for L, percore, np_ in classes:
        nk = -(-L // KCH)
        Lp = nk * KCH
        ngrp = -(-np_ // 2)
        cls = _Cls()
        cls.L, cls.Lp, cls.nk, cls.np_, cls.ngrp = L, Lp, nk, np_, ngrp
        cls.pairs_percore = percore
        cls.ht_off = ht_off
        cls.vv_base = vv_base
        cls.qk_offs = []
        cls.w_blocks = []

        def _grp_heads(c, g):
            pa = percore[c][2 * g]
            pb = percore[c][2 * g + 1] if 2 * g + 1 < np_ else None
            return (pa[1] if pa is not None else None,
                    pb[1] if pb is not None else None)

        for g in range(ngrp):
            hs = [_grp_heads(c, g) for c in range(NC)]

            def _compat(cand):
                for c, (ha, hb) in enumerate(hs):
                    cur = block_heads[c].get(cand)
                    if cur is not None:
                        if ha is not None and cur[0] is not None and cur[0] != ha:
                            return False
                        if hb is not None and cur[1] is not None and cur[1] != hb:
                            return False
                return True

            beta = None
            for cand in range(nblocks):
                if _compat(cand):
                    beta = cand
                    break
            if beta is None:
                beta = nblocks
                nblocks += 1
            for c, (ha, hb) in enumerate(hs):
                cur = block_heads[c].get(beta, (None, None))
                block_heads[c][beta] = (ha if ha is not None else cur[0],
                                        hb if hb is not None else cur[1])
            cls.w_blocks.append(beta)
            cls.qk_offs.append((qk_off, qk_off + Lp))
            qk_off += 2 * Lp
        cls.qslots = []
        q = 0
        while q < L:
            cls.qslots.append((q, min(QCH, L - q)))
            q += QCH
        ht_off += Lp
        vv_base += nk * np_
        out_classes.append(cls)

    slots = []
    eb_col = out_off = 0
    for ci, cls in enumerate(out_classes):
        for pl in range(cls.np_):
            for (qoff, qlen) in cls.qslots:
                sl = _Slot()
                sl.ci, sl.pl, sl.qoff, sl.qlen = ci, pl, qoff, qlen
                sl.nsub = -(-qlen // 128)
                sl.eb_col = eb_col
                sl.out_off = out_off
                eb_col += cls.nk * qlen
                out_off += sl.nsub
                slots.append(sl)

    # v strips appended after the qk blocks (contiguous per class)
    v_off = nblocks * 256
    for cls in out_classes:
        cls.v_off = v_off
        v_off += cls.np_ * 64

    sched = {
        "classes": out_classes,
        "slots": slots,
        "WTOT": v_off,
        "NW": nblocks,
        "HT_COLS": ht_off,
        "QKT_COLS": qk_off,
        "VCH": vv_base,
        "EB_COLS": eb_col,
        "NSUB": out_off,
        "lens": lens,
    }
    return sched


def _get_sched(lens_key):
    key = ("sched", lens_key)
    if key not in _CACHE:
        _CACHE[key] = _build_schedule(list(lens_key))
    return _CACHE[key]


# --------------------------------------------------------------------------
# device program
# --------------------------------------------------------------------------

def _build_nc(sched, use_bias):
    import concourse.mybir as mybir
    import concourse.tile as tile
    from concourse import bacc

    f16 = mybir.dt.float16

    nc = bacc.Bacc("TRN2", debug=False, num_devices=NC)
    # partition-major layouts: row p holds partition p's data contiguously
    # (large DMA descriptors). hT/W carry an extra 129th row for the bias.
    hT = nc.dram_tensor("hT", [129, 6 * sched["HT_COLS"]], f16,
                        kind="ExternalInput").ap()
    W = nc.dram_tensor("W", [129, 6 * sched["WTOT"]], f16,
                       kind="ExternalInput").ap()
    Eb = nc.dram_tensor("Eb", [128, sched["EB_COLS"]], f16,
                        kind="ExternalInput").ap()
    out = nc.dram_tensor("out", [128, sched["NSUB"] * 65], f16,
                         kind="ExternalOutput").ap()

    with tile.TileContext(nc) as tc:
        _emit_body(nc, tc, tile, mybir, hT, W, Eb, out, sched, use_bias)
    nc.compile()
    return nc


def _emit_body(nc, tc, tile, mybir, hT, W, Eb, out, sched, use_bias):
    f16 = mybir.dt.float16
    f32 = mybir.dt.float32
    Exp = mybir.ActivationFunctionType.Exp
    classes = sched["classes"]
    slots = sched["slots"]
    NW = sched["NW"]

    with (
        tc.tile_pool(name="per", bufs=1) as per,
        tc.tile_pool(name="ebp", bufs=3) as ebp,
        tc.tile_pool(name="ste", bufs=3) as ste,
        tc.tile_pool(name="stp", bufs=9) as stp,
        tc.tile_pool(name="osb", bufs=3) as osb,
        tc.tile_pool(name="psc", bufs=2, space="PSUM") as psc,
        tc.tile_pool(name="pat", bufs=2, space="PSUM") as pat,
        tc.tile_pool(name="pj", bufs=2, space="PSUM") as pj,
    ):
        # ---- persistent tiles ------------------------------------------
        # W: qk group blocks (256 cols: [qA|kB][kA|qB]) then v strips
        hT_sb = per.tile([128, 6, sched["HT_COLS"]], f16)
        W_sb = per.tile([128, 6, sched["WTOT"]], f16)
        qkT = per.tile([128, sched["QKT_COLS"]], f16)
        vv = per.tile([128, sched["VCH"], 65], f16)
        nc.vector.memset(vv[:, :, 64:65], 1.0)
        HTC = sched["HT_COLS"]
        WC = sched["WTOT"]
        if use_bias:
            hT_last = per.tile([1, 6, HTC], f16)
            W_last2 = per.tile([1, 6, WC], f16)
            nc.sync.dma_start(
                hT_last, hT[128:129, :].rearrange("o (i c) -> o i c", c=HTC)
            )
            nc.sync.dma_start(
                W_last2, W[128:129, :].rearrange("o (i c) -> o i c", c=WC)
            )

        # ---- upfront DMAs -----------------------------------------------
        # class 0 + W: per-ic chunks interleaved on SP so the first
        # projection chain starts after ~0.25MB; later classes: one strided
        # DMA each on the scalar queue (latency hidden by class-0 work)
        hT_src = hT[0:128, :].rearrange("p (i c) -> p i c", c=HTC)
        W_src = W[0:128, :].rearrange("p (i c) -> p i c", c=WC)

        c0 = classes[0].ht_off
        L0 = classes[0].Lp
        for ic in range(6):
            nc.sync.dma_start(
                hT_sb[:, ic, c0:c0 + L0], hT_src[:, ic, c0:c0 + L0]
            )
            nc.sync.dma_start(W_sb[:, ic], W_src[:, ic])
        for cls in classes[1:]:
            for ic in range(6):
                nc.sync.dma_start(
                    hT_sb[:, ic, cls.ht_off:cls.ht_off + cls.Lp],
                    hT_src[:, ic, cls.ht_off:cls.ht_off + cls.Lp],
                )

        # ---- projection job closures -----------------------------------
        def qk_half(cls, g, pss, lc, box, ics):
            # pss 0: W cols [0:128] = [qA|kB]; pss 1: [128:256] = [kA|qB]
            cw = min(QCH, cls.Lp - lc * QCH)
            c0 = cls.w_blocks[g] * 256 + 128 * pss
            dst = cls.qk_offs[g][pss] + lc * QCH
            if ics.start == 0:
                box["ps"] = pj.tile([128, QCH], f32, tag="pj", name="ps_qk")
            ps = box["ps"]
            last = ics.stop == 6
            for ic in ics:
                nc.tensor.matmul(
                    ps[:, :cw],
                    W_sb[:, ic, c0:c0 + 128],
                    hT_sb[:, ic, cls.ht_off + lc * QCH:
                          cls.ht_off + lc * QCH + cw],
                    start=(ic == 0),
                    stop=(ic == 5 and last and not use_bias),
                )
            if last:
                if use_bias:
                    nc.tensor.matmul(
                        ps[:, :cw],
                        W_last2[:, 0, c0:c0 + 128],
                        hT_last[:, 0, cls.ht_off + lc * QCH:
                                cls.ht_off + lc * QCH + cw],
                        start=False, stop=True,
                    )
                nc.vector.tensor_copy(qkT[:, dst:dst + cw], ps[:, :cw])

        def qk_job(cls, g, pss, lc):
            def run():
                qk_half(cls, g, pss, lc, {}, range(0, 6))
            return run

        def v_job(cls, kc):
            def run():
                npr = cls.np_
                n = npr * 64
                ps = pj.tile([128, QCH], f32, tag="pj", name="ps_v")
                for ic in range(6):
                    nc.tensor.matmul(
                        ps[:, :n],
                        hT_sb[:, ic, cls.ht_off + kc * KCH:
                              cls.ht_off + (kc + 1) * KCH],
                        W_sb[:, ic, cls.v_off:cls.v_off + n],
                        start=(ic == 0), stop=(ic == 5 and not use_bias),
                    )
                if use_bias:
                    nc.tensor.matmul(
                        ps[:, :n],
                        hT_last[:, 0, cls.ht_off + kc * KCH:
                                cls.ht_off + (kc + 1) * KCH],
                        W_last2[:, 0, cls.v_off:cls.v_off + n],
                        start=False, stop=True,
                    )
                nc.vector.tensor_copy(
                    vv[:, cls.vv_base + kc * npr:
                       cls.vv_base + (kc + 1) * npr, 0:64],
                    ps[:, :n].rearrange("p (h d) -> p h d", d=64),
                )
            return run

        def proj_jobs(ci):
            cls = classes[ci]
            jobs = []
            for g in range(cls.ngrp):
                for pss in range(2):
                    for lc in range(-(-cls.Lp // QCH)):
                        jobs.append(qk_job(cls, g, pss, lc))
            for kc in range(cls.nk):
                jobs.append(v_job(cls, kc))
            return jobs

        # ---- Eb prefetch ------------------------------------------------
        eb_tiles = {}

        def issue_eb(si):
            sl = slots[si]
            cls = classes[sl.ci]
            t = ebp.tile([128, 8, QCH], f16, tag="eb", name="eb")
            eb_tiles[si] = t
            nk1 = cls.nk // 2
            for (k0, k1) in ((0, nk1), (nk1, cls.nk)):
                if k1 <= k0:
                    continue
                c0 = sl.eb_col + k0 * sl.qlen
                nc.sync.dma_start(
                    t[:, k0:k1, :sl.qlen],
                    Eb[:, c0:c0 + (k1 - k0) * sl.qlen].rearrange(
                        "p (n q) -> p n q", q=sl.qlen
                    ),
                )

        # ---- prologue ---------------------------------------------------
        for job in proj_jobs(0):
            job()
        issue_eb(0)
        if len(slots) > 1:
            issue_eb(1)

        # ---- main loop --------------------------------------------------
        # software pipeline: scores/exp/mul for slot i run while the PV
        # matmuls for slot i-1 stream (each q-sub's PSUM accumulation group
        # is sequential in its own bank: start=True resets the whole bank)
        def emit_pv_all(pend):
            (pts, cls_p, pl_p, qlen_p, nsub_p, out_off_p) = pend
            ob = osb.tile([128, 4, 65], f16, tag="ob", name="ob")
            for sub in range(nsub_p):
                qn = min(128, qlen_p - sub * 128)
                att = pat.tile([128, 128], f32, tag="att", name="att",
                               padded_shape=[128, QCH])
                for kc in range(cls_p.nk):
                    nc.tensor.matmul(
                        att[0:qn, 0:65],
                        pts[kc // 2][:, kc % 2, sub * 128:sub * 128 + qn],
                        vv[:, cls_p.vv_base + kc * cls_p.np_ + pl_p, :],
                        start=(kc == 0), stop=(kc == cls_p.nk - 1),
                    )
                nc.vector.tensor_copy(ob[:, sub, :], att[:, 0:65])
            nc.sync.dma_start(
                out[:, out_off_p * 65:(out_off_p + nsub_p) * 65].rearrange(
                    "p (n x) -> p n x", x=65
                ),
                ob[:, :nsub_p, :],
            )

        si = 0
        pending = None
        for ci, cls in enumerate(classes):
            fillers = proj_jobs(ci + 1) if ci + 1 < len(classes) else []
            n_iters = cls.np_ * len(cls.qslots) * cls.nk
            stride = max(1, n_iters // max(1, len(fillers)))
            it = 0
            fi = 0
            for pl in range(cls.np_):
                for (qoff, qlen) in cls.qslots:
                    sl = slots[si]
                    eb = eb_tiles.pop(si)
                    g, half = pl // 2, pl % 2
                    off1, off2 = cls.qk_offs[g]
                    # half 0: q in T1[0:64],  k in T2[0:64]
                    # half 1: q in T2[64:128], k in T1[64:128]
                    p0 = 64 * half
                    koff = off1 if half else off2
                    qoff_t = off2 if half else off1

                    pts = []
                    for kg in range(0, cls.nk, 2):
                        n2 = min(2, cls.nk - kg)
                        sps = psc.tile([128, 2, QCH], f32, tag="sc",
                                       name="sps")
                        for j in range(n2):
                            kc = kg + j
                            nc.tensor.matmul(
                                sps[:, j, :qlen],
                                qkT[p0:p0 + 64,
                                    koff + kc * KCH:koff + (kc + 1) * KCH],
                                qkT[p0:p0 + 64,
                                    qoff_t + qoff:qoff_t + qoff + qlen],
                                start=True, stop=True,
                            )
                            if (fillers and fi < len(fillers)
                                    and it % stride == 0):
                                fillers[fi]()
                                fi += 1
                            it += 1
                        es = ste.tile([128, 2, QCH], f16, tag="es", name="es")
                        nc.scalar.activation(
                            es[:, :n2, :qlen], sps[:, :n2, :qlen], Exp,
                            scale=0.125,
                        )
                        pt = stp.tile([128, 2, QCH], f16, tag="pt", name="pt")
                        mul_eng = (nc.gpsimd if (kg // 2) % 4 == 3 else nc.vector)
                        mul_eng.tensor_mul(
                            pt[:, :n2, :qlen], es[:, :n2, :qlen],
                            eb[:, kg:kg + n2, :qlen]
                        )
                        pts.append(pt)
                    if pending is not None:
                        emit_pv_all(pending)
                    pending = (pts, cls, pl, qlen, sl.nsub, sl.out_off)
                    if si + 2 < len(slots):
                        issue_eb(si + 2)
                    si += 1
            while fi < len(fillers):
                fillers[fi]()
                fi += 1
        emit_pv_all(pending)


def _get_nc(lens_key, use_bias):
    key = ("nc", lens_key, use_bias)
    if key not in _CACHE:
        _CACHE[key] = _build_nc(_get_sched(lens_key), use_bias)
    return _CACHE[key]


# --------------------------------------------------------------------------
# host pack / unpack
# --------------------------------------------------------------------------

def prepare_in_maps(inputs):
    hidden = np.asarray(inputs["hidden_states"], np.float32)
    Wf = np.asarray(inputs["Wqkv_w"], np.float32)
    bvec = np.asarray(inputs["Wqkv_b"], np.float32)
    bias = np.asarray(inputs["bias"], np.float32)
    indices = np.asarray(inputs["indices"], np.int32)
    cu = np.asarray(inputs["cu_seqlens"], np.int64)
    lens = np.diff(cu).astype(np.int64)
    nnz = hidden.shape[0]

    # valid tokens must be the first L of each batch row-block
    expect = np.concatenate(
        [b * S + np.arange(l) for b, l in enumerate(lens)]
    ) if len(lens) == B else None
    contiguous = (
        expect is not None
        and indices.shape[0] == expect.shape[0]
        and np.array_equal(indices, expect)
    )
    if not contiguous:
        # fallback: dense full-length processing, scatter rows
        lens = np.full(B, S, np.int64)
        hp = np.zeros((B * S, HID), np.float32)
        hp[indices] = hidden
        tok = [hp[b * S:(b + 1) * S] for b in range(B)]
    else:
        tok = [hidden[cu[b]:cu[b + 1]] for b in range(B)]

    lens_key = tuple(int(x) for x in lens)
    sched = _get_sched(lens_key)
    use_bias = bool(np.any(bvec != 0.0))

    # 1/sqrt(D) is applied via the Exp activation's scale parameter
    Ws = Wf
    bs = bvec

    classes = sched["classes"]
    slots = sched["slots"]
    NW = sched["NW"]

    HTC = sched["HT_COLS"]

    WTOT = sched["WTOT"]

    def prep_core(c):
        hTa = np.zeros((HID + 1, HTC), np.float16)
        hTa[HID] = 1.0
        Wd = np.zeros((HID + 1, WTOT), np.float16)
        Ebd = np.zeros((128, sched["EB_COLS"]), np.float16)
        for cls in classes:
            batches = {p[0] for p in cls.pairs_percore[c] if p is not None}
            if batches:
                b0 = next(iter(batches))
                L = int(lens[b0])
                hTa[0:HID, cls.ht_off:cls.ht_off + L] = tok[b0].T
            for pl, pair in enumerate(cls.pairs_percore[c]):
                if pair is None:
                    continue
                _, h = pair
                beta = cls.w_blocks[pl // 2]
                half = pl % 2
                # qk block (256): [qA|kB][kA|qB]; v strip: v_off + pl*64
                qc0 = beta * 256 + (192 if half else 0)
                kc0 = beta * 256 + (64 if half else 128)
                vc0 = cls.v_off + pl * 64
                Wd[0:HID, qc0:qc0 + 64] = Ws[:, h * D:(h + 1) * D]
                Wd[0:HID, kc0:kc0 + 64] = Ws[:, HID + h * D:HID + (h + 1) * D]
                Wd[0:HID, vc0:vc0 + 64] = \
                    Ws[:, 2 * HID + h * D:2 * HID + (h + 1) * D]
                Wd[HID, qc0:qc0 + 64] = bs[h * D:(h + 1) * D]
                Wd[HID, kc0:kc0 + 64] = bs[HID + h * D:HID + (h + 1) * D]
                Wd[HID, vc0:vc0 + 64] = \
                    bs[2 * HID + h * D:2 * HID + (h + 1) * D]
        with np.errstate(under="ignore"):
            for sl in slots:
                cls = classes[sl.ci]
                pair = cls.pairs_percore[c][sl.pl]
                if pair is None:
                    continue
                b, h = pair
                L = int(lens[b])
                sub = bias[b, h, sl.qoff:sl.qoff + sl.qlen, 0:L]
                arr = np.zeros((cls.nk * KCH, sl.qlen), np.float16)
                arr[:L] = (np.exp(sub) * EB_SCALE).T.astype(np.float16)
                Ebd[:, sl.eb_col:sl.eb_col + cls.nk * sl.qlen] = (
                    arr.reshape(cls.nk, 128, sl.qlen)
                    .transpose(1, 0, 2).reshape(128, cls.nk * sl.qlen)
                )
        # partition-major repack: row p holds its 6 ic chunks contiguously
        hT_pm = np.zeros((129, 6 * HTC), np.float16)
        hT_pm[0:128] = (hTa[0:HID].reshape(6, 128, HTC)
                        .transpose(1, 0, 2).reshape(128, 6 * HTC))
        hT_pm[128, 0:HTC] = hTa[HID]
        W_pm = np.zeros((129, 6 * WTOT), np.float16)
        W_pm[0:128] = (Wd[0:HID].reshape(6, 128, WTOT)
                       .transpose(1, 0, 2).reshape(128, 6 * WTOT))
        W_pm[128, 0:WTOT] = Wd[HID]
        return {"hT": hT_pm, "W": W_pm, "Eb": Ebd}

    from concurrent.futures import ThreadPoolExecutor
    with ThreadPoolExecutor(max_workers=8) as ex:
        in_maps = list(ex.map(prep_core, range(NC)))

    meta = {
        "lens_key": lens_key,
        "cu": cu,
        "nnz": nnz,
        "contiguous": contiguous,
        "indices": indices,
    }
    return in_maps, meta, use_bias


def postprocess(results, meta):
    sched = _get_sched(meta["lens_key"])
    classes = sched["classes"]
    slots = sched["slots"]
    cu = meta["cu"]
    if meta["contiguous"]:
        out_full = np.zeros((meta["nnz"], HID), np.float32)
    else:
        out_full = np.zeros((B * S, HID), np.float32)
    for c in range(NC):
        o = np.asarray(results[c]["out"], np.float32)   # [128, NSUB*65]
        o = o.reshape(128, sched["NSUB"], 65).transpose(1, 0, 2)
        for sl in slots:
            cls = classes[sl.ci]
            pair = cls.pairs_percore[c][sl.pl]
            if pair is None:
                continue
            b, h = pair
            base = (cu[b] if meta["contiguous"] else b * S)
            for sub in range(sl.nsub):
                qn = min(128, sl.qlen - sub * 128)
                blk = o[sl.out_off + sub, :qn]
                att = blk[:, :64] / blk[:, 64:65]
                r0 = base + sl.qoff + sub * 128
                out_full[r0:r0 + qn, h * D:(h + 1) * D] = att
    if not meta["contiguous"]:
        out_full = out_full[meta["indices"]]
    return out_full


def _run_spmd(in_maps, meta, use_bias=True, trace=False):
    from concourse.bass_utils import run_bass_kernel_spmd
    return run_bass_kernel_spmd(
        _get_nc(meta["lens_key"], use_bias), in_maps,
        core_ids=list(range(NC)), trace=trace,
    )


def kernel(**inputs):
    in_maps, meta, use_bias = prepare_in_maps(inputs)
    res = _run_spmd(in_maps, meta, use_bias=use_bias)
    return postprocess(res.results, meta)


# revision 31
# speedup vs baseline: 1.0275x; 1.0175x over previous
"""Trainium2 Bass kernel: BertUnpadSelfAttention (B=8, S=1024, H=12, D=64).

Strategy (v2)
-------------
Shard by (batch, head) pairs instead of batch: the sequence lengths vary
(512..1024), so batch-per-core sharding makes every core pay for the
longest batch.  96 (b,h) pairs are grouped into length-classes and
bin-packed into an identical per-core slot schedule (same instruction
stream on all 8 cores; the host packs each core's pairs into the slots).
Masked work is skipped entirely: k-chunks and q-chunks beyond L are never
computed and their exp(bias) tiles are never streamed.

Device (per core, fp16 matmuls -> fp32 PSUM):
  * proj per pair: qk^T = W_qk^T @ hT -> [128 = q64|k64 dims, L tokens];
    v = hT^T @ W_v -> [token, head, d] with a ones column for the softmax
    denominator
  * per slot (pair, q-chunk<=512), per k-chunk of 128:
    scores^T [128k, qlen] = kT.T @ qT (one matmul), exp on ScalarE,
    multiply by host-precomputed exp(bias)*2^-4 tile on VectorE,
    then reverse-PV: att[q<=128, d|sum] += p^T.T @ [v|1]  (probs as the
    stationary operand: 65-row streams instead of 512 -> half the PE time)
  * proj of the next class is interleaved into the attention k-loop;
    PSUM->SBUF evacuations run on GpSimd (otherwise idle)
  * Eb streams via few large descriptor-rich DMAs issued from GpSimd

Host: pack per-core hT/W/Eb; post: divide by denominator, write rows
directly at cu_seqlens offsets (valid tokens are contiguous per batch).
"""

import numpy as np

B, S, H, D = 8, 1024, 12, 64
HID = H * D            # 768
NC = 8                 # cores
KCH = 128              # k chunk
QCH = 512              # max q chunk / moving free dim
EB_SCALE = 0.0625      # folded into exp(bias); cancels in softmax

_CACHE = {}


# --------------------------------------------------------------------------
# schedule
# --------------------------------------------------------------------------

class _Cls:
    __slots__ = ("L", "Lp", "nk", "np_", "ngrp", "ht_off", "vv_base",
                 "qk_offs", "w_blocks", "v_off", "qslots", "pairs_percore")


class _Slot:
    __slots__ = ("ci", "pl", "qoff", "qlen", "nsub", "eb_col", "out_off")


def _build_schedule(lens):
    """Uniform per-core slot schedule from the 8 sequence lengths."""
    lens = [int(x) for x in lens]
    assert len(lens) == B and all(0 < l <= S for l in lens)

    # merge batches with equal L into one class; if a core's slice of a
    # class would straddle two batches, fall back to one class per batch
    def mk_classes(group_by_len):
        if group_by_len:
            ls = sorted(set(lens), reverse=True)
            return [(L, [b for b in range(B) if lens[b] == L]) for L in ls]
        order = sorted(range(B), key=lambda b: -lens[b])
        return [(lens[b], [b]) for b in order]

    for group_by_len in (True, False):
        classes = []
        ok = True
        for L, batches in mk_classes(group_by_len):
            pairs = [(b, h) for b in batches for h in range(H)]
            np_ = -(-len(pairs) // NC)
            padded = pairs + [None] * (np_ * NC - len(pairs))
            percore = [padded[c * np_:(c + 1) * np_] for c in range(NC)]
            for c in range(NC):
                bs = {p[0] for p in percore[c] if p is not None}
                if len(bs) > 1:
                    ok = False
            classes.append((L, percore, np_))
        if ok:
            break
    assert ok, "schedule fallback failed"

    # W group-block dedupe: pairs are processed in groups of 2 (two swapped
    # projection passes [qA|kB], [kA|qB] so both operands of the scores
    # matmul share a base partition).  A group block can be reused if on
    # every core the (headA, headB) at that block matches (or is pad).
    block_heads = [dict() for _ in range(NC)]   # per core: block -> (hA, hB)
    nblocks = 0
    out_classes = []
    ht_off = vv_base = qk_off = 0
    for L, percore, np_ in classes:
        nk = -(-L // KCH)
        Lp = nk * KCH
        ngrp = -(-np_ // 2)
        cls = _Cls()
        cls.L, cls.Lp, cls.nk, cls.np_, cls.ngrp = L, Lp, nk, np_, ngrp
        cls.pairs_percore = percore
        cls.ht_off = ht_off
        cls.vv_base = vv_base
        cls.qk_offs = []
        cls.w_blocks = []

        def _grp_heads(c, g):
            pa = percore[c][2 * g]
            pb = percore[c][2 * g + 1] if 2 * g + 1 < np_ else None
            return (pa[1] if pa is not None else None,
                    pb[1] if pb is not None else None)

        for g in range(ngrp):
            hs = [_grp_heads(c, g) for c in range(NC)]

            def _compat(cand):
                for c, (ha, hb) in enumerate(hs):
                    cur = block_heads[c].get(cand)
                    if cur is not None:
                        if ha is not None and cur[0] is not None and cur[0] != ha:
                            return False
                        if hb is not None and cur[1] is not None and cur[1] != hb:
                            return False
                return True

            beta = None
            for cand in range(nblocks):
                if _compat(cand):
                    beta = cand
                    break
            if beta is None:
                beta = nblocks
                nblocks += 1
            for c, (ha, hb) in enumerate(hs):
                cur = block_heads[c].get(beta, (None, None))
                block_heads[c][beta] = (ha if ha is not None else cur[0],
                                        hb if hb is not None else cur[1])
            cls.w_blocks.append(beta)
            cls.qk_offs.append((qk_off, qk_off + Lp))
            qk_off += 2 * Lp
        cls.qslots = []
        q = 0
        while q < L:
            cls.qslots.append((q, min(QCH, L - q)))
            q += QCH
        ht_off += Lp
        vv_base += nk * np_
        out_classes.append(cls)

    slots = []
    eb_col = out_off = 0
    for ci, cls in enumerate(out_classes):
        for pl in range(cls.np_):
            for (qoff, qlen) in cls.qslots:
                sl = _Slot()
                sl.ci, sl.pl, sl.qoff, sl.qlen = ci, pl, qoff, qlen
                sl.nsub = -(-qlen // 128)
                sl.eb_col = eb_col
                sl.out_off = out_off
                eb_col += cls.nk * qlen
                out_off += sl.nsub
                slots.append(sl)

    # v strips appended after the qk blocks (contiguous per class)
    v_off = nblocks * 256
    for cls in out_classes:
        cls.v_off = v_off
        v_off += cls.np_ * 64

    sched = {
        "classes": out_classes,
        "slots": slots,
        "WTOT": v_off,
        "NW": nblocks,
        "HT_COLS": ht_off,
        "QKT_COLS": qk_off,
        "VCH": vv_base,
        "EB_COLS": eb_col,
        "NSUB": out_off,
        "lens": lens,
    }
    return sched


def _get_sched(lens_key):
    key = ("sched", lens_key)
    if key not in _CACHE:
        _CACHE[key] = _build_schedule(list(lens_key))
    return _CACHE[key]


# --------------------------------------------------------------------------
# device program
# --------------------------------------------------------------------------

def _build_nc(sched, use_bias):
    import concourse.mybir as mybir
    import concourse.tile as tile
    from concourse import bacc

    f16 = mybir.dt.float16

    nc = bacc.Bacc("TRN2", debug=False, num_devices=NC)
    # partition-major layouts: row p holds partition p's data contiguously
    # (large DMA descriptors). hT/W carry an extra 129th row for the bias.
    hT = nc.dram_tensor("hT", [129, 6 * sched["HT_COLS"]], f16,
                        kind="ExternalInput").ap()
    W = nc.dram_tensor("W", [129, 6 * sched["WTOT"]], f16,
                       kind="ExternalInput").ap()
    Eb = nc.dram_tensor("Eb", [128, sched["EB_COLS"]], f16,
                        kind="ExternalInput").ap()
    out = nc.dram_tensor("out", [128, sched["NSUB"] * 65], f16,
                         kind="ExternalOutput").ap()

    with tile.TileContext(nc) as tc:
        _emit_body(nc, tc, tile, mybir, hT, W, Eb, out, sched, use_bias)
    nc.compile()
    return nc


def _emit_body(nc, tc, tile, mybir, hT, W, Eb, out, sched, use_bias):
    f16 = mybir.dt.float16
    f32 = mybir.dt.float32
    Exp = mybir.ActivationFunctionType.Exp
    classes = sched["classes"]
    slots = sched["slots"]
    NW = sched["NW"]

    with (
        tc.tile_pool(name="per", bufs=1) as per,
        tc.tile_pool(name="ebp", bufs=4) as ebp,
        tc.tile_pool(name="ste", bufs=4) as ste,
        tc.tile_pool(name="stp", bufs=9) as stp,
        tc.tile_pool(name="osb", bufs=3) as osb,
        tc.tile_pool(name="psc", bufs=2, space="PSUM") as psc,
        tc.tile_pool(name="pat", bufs=2, space="PSUM") as pat,
        tc.tile_pool(name="pj", bufs=2, space="PSUM") as pj,
    ):
        # ---- persistent tiles ------------------------------------------
        # W: qk group blocks (256 cols: [qA|kB][kA|qB]) then v strips
        hT_sb = per.tile([128, 6, sched["HT_COLS"]], f16)
        W_sb = per.tile([128, 6, sched["WTOT"]], f16)
        qkT = per.tile([128, sched["QKT_COLS"]], f16)
        vv = per.tile([128, sched["VCH"], 65], f16)
        nc.vector.memset(vv[:, :, 64:65], 1.0)
        HTC = sched["HT_COLS"]
        WC = sched["WTOT"]
        if use_bias:
            hT_last = per.tile([1, 6, HTC], f16)
            W_last2 = per.tile([1, 6, WC], f16)
            nc.sync.dma_start(
                hT_last, hT[128:129, :].rearrange("o (i c) -> o i c", c=HTC)
            )
            nc.sync.dma_start(
                W_last2, W[128:129, :].rearrange("o (i c) -> o i c", c=WC)
            )

        # ---- upfront DMAs -----------------------------------------------
        # class 0 + W: per-ic chunks interleaved on SP so the first
        # projection chain starts after ~0.25MB; later classes: one strided
        # DMA each on the scalar queue (latency hidden by class-0 work)
        hT_src = hT[0:128, :].rearrange("p (i c) -> p i c", c=HTC)
        W_src = W[0:128, :].rearrange("p (i c) -> p i c", c=WC)

        c0 = classes[0].ht_off
        L0 = classes[0].Lp
        for ic in range(6):
            nc.sync.dma_start(
                hT_sb[:, ic, c0:c0 + L0], hT_src[:, ic, c0:c0 + L0]
            )
            nc.sync.dma_start(W_sb[:, ic], W_src[:, ic])
        for cls in classes[1:]:
            for ic in range(6):
                nc.sync.dma_start(
                    hT_sb[:, ic, cls.ht_off:cls.ht_off + cls.Lp],
                    hT_src[:, ic, cls.ht_off:cls.ht_off + cls.Lp],
                )

        # ---- projection job closures -----------------------------------
        def qk_half(cls, g, pss, lc, box, ics):
            # pss 0: W cols [0:128] = [qA|kB]; pss 1: [128:256] = [kA|qB]
            cw = min(QCH, cls.Lp - lc * QCH)
            c0 = cls.w_blocks[g] * 256 + 128 * pss
            dst = cls.qk_offs[g][pss] + lc * QCH
            if ics.start == 0:
                box["ps"] = pj.tile([128, QCH], f32, tag="pj", name="ps_qk")
            ps = box["ps"]
            last = ics.stop == 6
            for ic in ics:
                nc.tensor.matmul(
                    ps[:, :cw],
                    W_sb[:, ic, c0:c0 + 128],
                    hT_sb[:, ic, cls.ht_off + lc * QCH:
                          cls.ht_off + lc * QCH + cw],
                    start=(ic == 0),
                    stop=(ic == 5 and last and not use_bias),
                )
            if last:
                if use_bias:
                    nc.tensor.matmul(
                        ps[:, :cw],
                        W_last2[:, 0, c0:c0 + 128],
                        hT_last[:, 0, cls.ht_off + lc * QCH:
                                cls.ht_off + lc * QCH + cw],
                        start=False, stop=True,
                    )
                nc.vector.tensor_copy(qkT[:, dst:dst + cw], ps[:, :cw])

        def qk_job(cls, g, pss, lc):
            def run():
                qk_half(cls, g, pss, lc, {}, range(0, 6))
            return run

        def v_job(cls, kc):
            def run():
                npr = cls.np_
                n = npr * 64
                ps = pj.tile([128, QCH], f32, tag="pj", name="ps_v")
                for ic in range(6):
                    nc.tensor.matmul(
                        ps[:, :n],
                        hT_sb[:, ic, cls.ht_off + kc * KCH:
                              cls.ht_off + (kc + 1) * KCH],
                        W_sb[:, ic, cls.v_off:cls.v_off + n],
                        start=(ic == 0), stop=(ic == 5 and not use_bias),
                    )
                if use_bias:
                    nc.tensor.matmul(
                        ps[:, :n],
                        hT_last[:, 0, cls.ht_off + kc * KCH:
                                cls.ht_off + (kc + 1) * KCH],
                        W_last2[:, 0, cls.v_off:cls.v_off + n],
                        start=False, stop=True,
                    )
                nc.vector.tensor_copy(
                    vv[:, cls.vv_base + kc * npr:
                       cls.vv_base + (kc + 1) * npr, 0:64],
                    ps[:, :n].rearrange("p (h d) -> p h d", d=64),
                )
            return run

        def proj_jobs(ci):
            cls = classes[ci]
            jobs = []
            for g in range(cls.ngrp):
                for pss in range(2):
                    for lc in range(-(-cls.Lp // QCH)):
                        jobs.append(qk_job(cls, g, pss, lc))
            for kc in range(cls.nk):
                jobs.append(v_job(cls, kc))
            return jobs

        # ---- Eb prefetch ------------------------------------------------
        eb_tiles = {}

        def issue_eb(si):
            sl = slots[si]
            cls = classes[sl.ci]
            t = ebp.tile([128, 8, QCH], f16, tag="eb", name="eb")
            eb_tiles[si] = t
            nk1 = cls.nk // 2
            for (k0, k1) in ((0, nk1), (nk1, cls.nk)):
                if k1 <= k0:
                    continue
                c0 = sl.eb_col + k0 * sl.qlen
                nc.sync.dma_start(
                    t[:, k0:k1, :sl.qlen],
                    Eb[:, c0:c0 + (k1 - k0) * sl.qlen].rearrange(
                        "p (n q) -> p n q", q=sl.qlen
                    ),
                )

        # ---- prologue ---------------------------------------------------
        for job in proj_jobs(0):
            job()
        issue_eb(0)
        if len(slots) > 1:
            issue_eb(1)

        # ---- main loop --------------------------------------------------
        # software pipeline: scores/exp/mul for slot i run while the PV
        # matmuls for slot i-1 stream (each q-sub's PSUM accumulation group
        # is sequential in its own bank: start=True resets the whole bank)
        def emit_pv_all(pend):
            (pts, cls_p, pl_p, qlen_p, nsub_p, out_off_p) = pend
            ob = osb.tile([128, 4, 65], f16, tag="ob", name="ob")
            for sub in range(nsub_p):
                qn = min(128, qlen_p - sub * 128)
                att = pat.tile([128, 128], f32, tag="att", name="att",
                               padded_shape=[128, QCH])
                for kc in range(cls_p.nk):
                    nc.tensor.matmul(
                        att[0:qn, 0:65],
                        pts[kc // 2][:, kc % 2, sub * 128:sub * 128 + qn],
                        vv[:, cls_p.vv_base + kc * cls_p.np_ + pl_p, :],
                        start=(kc == 0), stop=(kc == cls_p.nk - 1),
                    )
                nc.vector.tensor_copy(ob[:, sub, :], att[:, 0:65])
            nc.sync.dma_start(
                out[:, out_off_p * 65:(out_off_p + nsub_p) * 65].rearrange(
                    "p (n x) -> p n x", x=65
                ),
                ob[:, :nsub_p, :],
            )

        si = 0
        pending = None
        for ci, cls in enumerate(classes):
            fillers = proj_jobs(ci + 1) if ci + 1 < len(classes) else []
            n_iters = cls.np_ * len(cls.qslots) * cls.nk
            stride = max(1, n_iters // max(1, len(fillers)))
            it = 0
            fi = 0
            for pl in range(cls.np_):
                for (qoff, qlen) in cls.qslots:
                    sl = slots[si]
                    eb = eb_tiles.pop(si)
                    g, half = pl // 2, pl % 2
                    off1, off2 = cls.qk_offs[g]
                    # half 0: q in T1[0:64],  k in T2[0:64]
                    # half 1: q in T2[64:128], k in T1[64:128]
                    p0 = 64 * half
                    koff = off1 if half else off2
                    qoff_t = off2 if half else off1

                    pts = []
                    for kg in range(0, cls.nk, 2):
                        n2 = min(2, cls.nk - kg)
                        sps = psc.tile([128, 2, QCH], f32, tag="sc",
                                       name="sps")
                        for j in range(n2):
                            kc = kg + j
                            nc.tensor.matmul(
                                sps[:, j, :qlen],
                                qkT[p0:p0 + 64,
                                    koff + kc * KCH:koff + (kc + 1) * KCH],
                                qkT[p0:p0 + 64,
                                    qoff_t + qoff:qoff_t + qoff + qlen],
                                start=True, stop=True,
                            )
                            if (fillers and fi < len(fillers)
                                    and it % stride == 0):
                                fillers[fi]()
                                fi += 1
                            it += 1
                        es = ste.tile([128, 2, QCH], f16, tag="es", name="es")
                        nc.scalar.activation(
                            es[:, :n2, :qlen], sps[:, :n2, :qlen], Exp,
                            scale=0.125,
                        )
                        pt = stp.tile([128, 2, QCH], f16, tag="pt", name="pt")
                        mul_eng = (nc.gpsimd if (kg // 2) % 4 == 3 else nc.vector)
                        mul_eng.tensor_mul(
                            pt[:, :n2, :qlen], es[:, :n2, :qlen],
                            eb[:, kg:kg + n2, :qlen]
                        )
                        pts.append(pt)
                    if pending is not None:
                        emit_pv_all(pending)
                    pending = (pts, cls, pl, qlen, sl.nsub, sl.out_off)
                    if si + 2 < len(slots):
                        issue_eb(si + 2)
                    si += 1
            while fi < len(fillers):
                fillers[fi]()
                fi += 1
        emit_pv_all(pending)


def _get_nc(lens_key, use_bias):
    key = ("nc", lens_key, use_bias)
    if key not in _CACHE:
        _CACHE[key] = _build_nc(_get_sched(lens_key), use_bias)
    return _CACHE[key]


# --------------------------------------------------------------------------
# host pack / unpack
# --------------------------------------------------------------------------

def prepare_in_maps(inputs):
    hidden = np.asarray(inputs["hidden_states"], np.float32)
    Wf = np.asarray(inputs["Wqkv_w"], np.float32)
    bvec = np.asarray(inputs["Wqkv_b"], np.float32)
    bias = np.asarray(inputs["bias"], np.float32)
    indices = np.asarray(inputs["indices"], np.int32)
    cu = np.asarray(inputs["cu_seqlens"], np.int64)
    lens = np.diff(cu).astype(np.int64)
    nnz = hidden.shape[0]

    # valid tokens must be the first L of each batch row-block
    expect = np.concatenate(
        [b * S + np.arange(l) for b, l in enumerate(lens)]
    ) if len(lens) == B else None
    contiguous = (
        expect is not None
        and indices.shape[0] == expect.shape[0]
        and np.array_equal(indices, expect)
    )
    if not contiguous:
        # fallback: dense full-length processing, scatter rows
        lens = np.full(B, S, np.int64)
        hp = np.zeros((B * S, HID), np.float32)
        hp[indices] = hidden
        tok = [hp[b * S:(b + 1) * S] for b in range(B)]
    else:
        tok = [hidden[cu[b]:cu[b + 1]] for b in range(B)]

    lens_key = tuple(int(x) for x in lens)
    sched = _get_sched(lens_key)
    use_bias = bool(np.any(bvec != 0.0))

    # 1/sqrt(D) is applied via the Exp activation's scale parameter
    Ws = Wf
    bs = bvec

    classes = sched["classes"]
    slots = sched["slots"]
    NW = sched["NW"]

    HTC = sched["HT_COLS"]

    WTOT = sched["WTOT"]

    def prep_core(c):
        hTa = np.zeros((HID + 1, HTC), np.float16)
        hTa[HID] = 1.0
        Wd = np.zeros((HID + 1, WTOT), np.float16)
        Ebd = np.zeros((128, sched["EB_COLS"]), np.float16)
        for cls in classes:
            batches = {p[0] for p in cls.pairs_percore[c] if p is not None}
            if batches:
                b0 = next(iter(batches))
                L = int(lens[b0])
                hTa[0:HID, cls.ht_off:cls.ht_off + L] = tok[b0].T
            for pl, pair in enumerate(cls.pairs_percore[c]):
                if pair is None:
                    continue
                _, h = pair
                beta = cls.w_blocks[pl // 2]
                half = pl % 2
                # qk block (256): [qA|kB][kA|qB]; v strip: v_off + pl*64
                qc0 = beta * 256 + (192 if half else 0)
                kc0 = beta * 256 + (64 if half else 128)
                vc0 = cls.v_off + pl * 64
                Wd[0:HID, qc0:qc0 + 64] = Ws[:, h * D:(h + 1) * D]
                Wd[0:HID, kc0:kc0 + 64] = Ws[:, HID + h * D:HID + (h + 1) * D]
                Wd[0:HID, vc0:vc0 + 64] = \
                    Ws[:, 2 * HID + h * D:2 * HID + (h + 1) * D]
                Wd[HID, qc0:qc0 + 64] = bs[h * D:(h + 1) * D]
                Wd[HID, kc0:kc0 + 64] = bs[HID + h * D:HID + (h + 1) * D]
                Wd[HID, vc0:vc0 + 64] = \
                    bs[2 * HID + h * D:2 * HID + (h + 1) * D]
        with np.errstate(under="ignore"):
            for sl in slots:
                cls = classes[sl.ci]
                pair = cls.pairs_percore[c][sl.pl]
                if pair is None:
                    continue
                b, h = pair
                L = int(lens[b])
                sub = bias[b, h, sl.qoff:sl.qoff + sl.qlen, 0:L]
                arr = np.zeros((cls.nk * KCH, sl.qlen), np.float16)
                arr[:L] = (np.exp(sub) * EB_SCALE).T.astype(np.float16)
                Ebd[:, sl.eb_col:sl.eb_col + cls.nk * sl.qlen] = (
                    arr.reshape(cls.nk, 128, sl.qlen)
                    .transpose(1, 0, 2).reshape(128, cls.nk * sl.qlen)
                )
        # partition-major repack: row p holds its 6 ic chunks contiguously
        hT_pm = np.zeros((129, 6 * HTC), np.float16)
        hT_pm[0:128] = (hTa[0:HID].reshape(6, 128, HTC)
                        .transpose(1, 0, 2).reshape(128, 6 * HTC))
        hT_pm[128, 0:HTC] = hTa[HID]
        W_pm = np.zeros((129, 6 * WTOT), np.float16)
        W_pm[0:128] = (Wd[0:HID].reshape(6, 128, WTOT)
                       .transpose(1, 0, 2).reshape(128, 6 * WTOT))
        W_pm[128, 0:WTOT] = Wd[HID]
        return {"hT": hT_pm, "W": W_pm, "Eb": Ebd}

    from concurrent.futures import ThreadPoolExecutor
    with ThreadPoolExecutor(max_workers=8) as ex:
        in_maps = list(ex.map(prep_core, range(NC)))

    meta = {
        "lens_key": lens_key,
        "cu": cu,
        "nnz": nnz,
        "contiguous": contiguous,
        "indices": indices,
    }
    return in_maps, meta, use_bias


def postprocess(results, meta):
    sched = _get_sched(meta["lens_key"])
    classes = sched["classes"]
    slots = sched["slots"]
    cu = meta["cu"]
    if meta["contiguous"]:
        out_full = np.zeros((meta["nnz"], HID), np.float32)
    else:
        out_full = np.zeros((B * S, HID), np.float32)
    for c in range(NC):
        o = np.asarray(results[c]["out"], np.float32)   # [128, NSUB*65]
        o = o.reshape(128, sched["NSUB"], 65).transpose(1, 0, 2)
        for sl in slots:
            cls = classes[sl.ci]
            pair = cls.pairs_percore[c][sl.pl]
            if pair is None:
                continue
            b, h = pair
            base = (cu[b] if meta["contiguous"] else b * S)
            for sub in range(sl.nsub):
                qn = min(128, sl.qlen - sub * 128)
                blk = o[sl.out_off + sub, :qn]
                att = blk[:, :64] / blk[:, 64:65]
                r0 = base + sl.qoff + sub * 128
                out_full[r0:r0 + qn, h * D:(h + 1) * D] = att
    if not meta["contiguous"]:
        out_full = out_full[meta["indices"]]
    return out_full


def _run_spmd(in_maps, meta, use_bias=True, trace=False):
    from concourse.bass_utils import run_bass_kernel_spmd
    return run_bass_kernel_spmd(
        _get_nc(meta["lens_key"], use_bias), in_maps,
        core_ids=list(range(NC)), trace=trace,
    )


def kernel(**inputs):
    in_maps, meta, use_bias = prepare_in_maps(inputs)
    res = _run_spmd(in_maps, meta, use_bias=use_bias)
    return postprocess(res.results, meta)
